# revision 12
# baseline (speedup 1.0000x reference)
"""MicrostateDGFLayer Trainium2 kernel.

Data-parallel over batch B=8 across 8 NeuronCores (one batch element per
core). Per core:
- pairwise sq-distances via bf16 PE matmuls (exact-diagonal trick: +8*I
  bump on the distance psum diagonal plus a min-clamp of the diagonal
  block pins adj[n,n] to its analytic value, making bf16 cancellation
  error there irrelevant; off-diagonal distances are O(100) so bf16
  noise vanishes under exp),
- per-head Gaussian kernel on ACT with gate/head-count folded into the
  exp bias (written directly as bf16 "eb" tiles),
- gated fusion with prev_adj on DVE (f32 adj output),
- graph propagation h = adj @ (x@W.T) with NO on-chip transposes: the
  Gaussian part of adj is symmetric, so its transposed blocks are other
  row-tiles' column slices (eb_j[:, i*128:...]); the asymmetric prev
  part streams host-pretransposed (1-gate)*prev.T in bf16. hT
  accumulates in a persistent [128 d, 2048 n] psum (4 banks), one
  weight load (xW_j) + 32 N=128 matmuls per row tile.
- tail: bias add, PE transpose back to row-major, ELU+residual+LN.
"""

import math
from contextlib import ExitStack

import numpy as np

B, N, D, H = 8, 2048, 128, 8
P = 128
NT = N // P          # 16 row tiles per core
QW = 512             # dist psum computed in [128, 512] quarters (1 bank)
LN_EPS = 1e-5
KERN_EPS = 1e-6
BUMP = 8.0           # diagonal psum bump; > any bf16 matmul error there

_CACHE: dict = {}


def _build_program(scales, gcs, one_minus_gate):
    """Build + compile the SPMD Bass program. scales[k] = 2/denom_k,
    gcs[k] = gate*count_k/H (both baked as instruction immediates)."""
    import concourse.bacc as bacc
    import concourse.tile as tile
    from concourse import mybir

    U = len(scales)
    f32 = mybir.dt.float32
    bf16 = mybir.dt.bfloat16
    Alu = mybir.AluOpType
    Act = mybir.ActivationFunctionType
    MMW = min(QW, N)
    import ml_dtypes
    gc_sum = float(sum(gcs))
    diag_delta = gc_sum - float(np.float32(ml_dtypes.bfloat16(gc_sum)))

    nc = bacc.Bacc("TRN2", target_bir_lowering=False, debug=False,
                   num_devices=B)

    xTb_d = nc.dram_tensor("xTb", [D, N], bf16, kind="ExternalInput").ap()
    xm1_d = nc.dram_tensor("xm1", [N, D], f32, kind="ExternalInput").ap()
    prev_d = nc.dram_tensor("prev", [N, N], f32, kind="ExternalInput").ap()
    pvtb_d = nc.dram_tensor("pvtb", [N, N], bf16, kind="ExternalInput").ap()
    negsq_d = nc.dram_tensor("negsq", [N], bf16, kind="ExternalInput").ap()
    biasv_d = nc.dram_tensor("biasv", [U, N], f32, kind="ExternalInput").ap()
    WTb_d = nc.dram_tensor("WTb", [D, D], bf16, kind="ExternalInput").ap()
    bump_d = nc.dram_tensor("bump4", [4, P, QW], bf16,
                            kind="ExternalInput").ap()
    idb_d = nc.dram_tensor("identb", [P, P], bf16, kind="ExternalInput").ap()
    idf_d = nc.dram_tensor("identf", [P, P], f32, kind="ExternalInput").ap()
    bvec_d = nc.dram_tensor("bvec", [D], f32, kind="ExternalInput").ap()
    gam_d = nc.dram_tensor("gamma_rep", [P, D], f32, kind="ExternalInput").ap()
    bet_d = nc.dram_tensor("beta_rep", [P, D], f32, kind="ExternalInput").ap()
    adj_d = nc.dram_tensor("adj", [N, N], f32, kind="ExternalOutput").ap()
    out_d = nc.dram_tensor("out", [N, D], f32, kind="ExternalOutput").ap()

    with tile.TileContext(nc) as tc, ExitStack() as ctx:
        consts = ctx.enter_context(tc.tile_pool(name="consts", bufs=1))
        prev_p = ctx.enter_context(tc.tile_pool(name="prev", bufs=3))
        pvtb_p = ctx.enter_context(tc.tile_pool(name="pvtb", bufs=3))
        adj_p = ctx.enter_context(tc.tile_pool(name="adjp", bufs=3))
        eb_p = ctx.enter_context(tc.tile_pool(name="ebp", bufs=3))
        htb_p = ctx.enter_context(tc.tile_pool(name="htb", bufs=2))
        epi_p = ctx.enter_context(tc.tile_pool(name="epi", bufs=3))
        ps_dist = ctx.enter_context(
            tc.tile_pool(name="ps_dist", bufs=3, space="PSUM"))
        ps_h = ctx.enter_context(
            tc.tile_pool(name="ps_h", bufs=1, space="PSUM"))
        ps_tph = ctx.enter_context(
            tc.tile_pool(name="ps_tph", bufs=1, space="PSUM"))

        # ---- constants ----
        xTb = consts.tile([D, N], bf16)
        nc.sync.dma_start(out=xTb, in_=xTb_d)
        WTb = consts.tile([D, D], bf16)
        nc.sync.dma_start(out=WTb, in_=WTb_d)
        xm1 = consts.tile([P, NT, D], f32)
        nc.sync.dma_start(out=xm1, in_=xm1_d.rearrange("(j p) d -> p j d", p=P))
        negsq = consts.tile([1, N], bf16)
        nc.sync.dma_start(out=negsq, in_=negsq_d.rearrange("(o n) -> o n", o=1))
        biasv = consts.tile([P, U, NT], f32)
        nc.sync.dma_start(out=biasv,
                          in_=biasv_d.rearrange("u (j p) -> p u j", p=P))
        bump = consts.tile([P, 4, QW], bf16)
        nc.sync.dma_start(out=bump, in_=bump_d.rearrange("v p n -> p v n"))
        identb = consts.tile([P, P], bf16)
        nc.sync.dma_start(out=identb, in_=idb_d)
        identf = consts.tile([P, P], f32)
        nc.sync.dma_start(out=identf, in_=idf_d)
        bvec = consts.tile([D, 1], f32)
        nc.sync.dma_start(out=bvec, in_=bvec_d.rearrange("(d o) -> d o", o=1))
        gam = consts.tile([P, D], f32)
        nc.sync.dma_start(out=gam, in_=gam_d)
        bet = consts.tile([P, D], f32)
        nc.sync.dma_start(out=bet, in_=bet_d)
        ones1 = consts.tile([1, P], bf16)
        nc.vector.memset(ones1, 1.0)
        eps_t = consts.tile([P, 1], f32)
        nc.vector.memset(eps_t, LN_EPS)

        # ---- xWb = bf16(x @ W.T), row-major [m, d] chunks ----
        xWb = consts.tile([P, N], bf16)
        for mc in range(NT):
            pt = ps_dist.tile([P, P], f32, tag="ps")
            nc.tensor.matmul(pt, xTb[:, mc * P:(mc + 1) * P], WTb,
                             start=True, stop=True)
            nc.scalar.activation(xWb[:, mc * P:(mc + 1) * P], pt, Act.Copy)

        # persistent hT accumulator [d, n] over the whole row sweep
        ht = ps_h.tile([P, N], f32)

        for j in range(NT):
            rows = slice(j * P, (j + 1) * P)
            gd = j * P                      # diagonal block column offset
            prev_t = prev_p.tile([P, N], f32)
            nc.sync.dma_start(out=prev_t, in_=prev_d[rows, :])
            pvtb_t = pvtb_p.tile([P, N], bf16)
            nc.sync.dma_start(out=pvtb_t, in_=pvtb_d[rows, :])
            adj_t = adj_p.tile([P, N], f32)
            eb_t = eb_p.tile([P, N], bf16)

            for q in range(N // MMW):
                c0 = q * MMW
                ps = ps_dist.tile([P, MMW], f32, tag="ps")
                # psum = x_j@x.T - sq_m/2 (+BUMP on diag); ACT scale doubles
                nc.tensor.matmul(ps, xTb[:, rows], xTb[:, c0:c0 + MMW],
                                 start=True, stop=False)
                has_diag = c0 <= gd < c0 + MMW
                nc.tensor.matmul(ps, ones1, negsq[:, c0:c0 + MMW],
                                 start=False, stop=not has_diag)
                if has_diag:
                    v = (gd % MMW) // P
                    nc.tensor.matmul(ps, identb, bump[:, v, :MMW],
                                     start=False, stop=True)
                cols = slice(c0, c0 + MMW)
                if U == 1:
                    nc.scalar.activation(eb_t[:, cols], ps, Act.Exp,
                                         bias=biasv[:, 0, j:j + 1],
                                         scale=scales[0])
                else:
                    accf = epi_p.tile([P, MMW], f32, tag="accf")
                    for u in range(U):
                        e_u = epi_p.tile([P, MMW], f32, tag="e_u")
                        dst = accf if u == 0 else e_u
                        nc.scalar.activation(dst, ps, Act.Exp,
                                             bias=biasv[:, u, j:j + 1],
                                             scale=scales[u])
                        if has_diag:
                            dc = gd % MMW
                            nc.vector.tensor_scalar(dst[:, dc:dc + P],
                                                    dst[:, dc:dc + P],
                                                    gcs[u], None, Alu.min)
                        if u > 0:
                            nc.vector.tensor_add(accf, accf, e_u)
                    nc.vector.tensor_copy(eb_t[:, cols], accf)
                if U == 1 and has_diag:
                    dc = gd % MMW
                    nc.vector.tensor_scalar(eb_t[:, gd:gd + P],
                                            eb_t[:, gd:gd + P],
                                            gcs[0], None, Alu.min)
                # adj = (prev * (1-gate)) + eb
                nc.vector.scalar_tensor_tensor(
                    adj_t[:, cols], prev_t[:, cols], one_minus_gate,
                    eb_t[:, cols], Alu.mult, Alu.add)

            # eb's diagonal is bf16(sum gc) — restore the exact f32 value
            # on adj's diagonal with a delta*I correction.
            nc.vector.scalar_tensor_tensor(
                adj_t[:, gd:gd + P], identf, diag_delta,
                adj_t[:, gd:gd + P], Alu.mult, Alu.add)
            nc.sync.dma_start(out=adj_d[rows, :], in_=adj_t)

            # hT[d, n] += xW_j.T @ (sym-part + prevT-part) for chunk j.
            # eb_j[:, i*128:...] IS the transposed block of the symmetric
            # Gaussian part; pvtb_j is host-pretransposed (1-g)*prev.T.
            for i in range(NT):
                isl = slice(i * P, (i + 1) * P)
                # start=True zeroes the whole 2KB psum bank (4 of these
                # [128,128] regions): only the first matmul of each bank
                # may set it; the bank's pending-zero initializes the
                # other regions' first writes.
                nc.tensor.matmul(ht[:, isl], xWb[:, rows], eb_t[:, isl],
                                 start=(j == 0 and i % 4 == 0), stop=False,
                                 skip_group_check=True)
                nc.tensor.matmul(ht[:, isl], xWb[:, rows], pvtb_t[:, isl],
                                 start=False, stop=(j == NT - 1),
                                 skip_group_check=True)

        # ---- tail: bias, transpose back, ELU + residual + LayerNorm ----
        for i in range(NT):
            isl = slice(i * P, (i + 1) * P)
            htb = htb_p.tile([P, P], f32)
            nc.scalar.activation(htb, ht[:, isl], Act.Identity, bias=bvec)
            hp = ps_tph.tile([P, P], f32, tag="tph")
            nc.tensor.transpose(hp, htb, identf)
            # a = relu(h) + exp(min(h,0)) + (x-1)  == elu(h) + x
            m0 = epi_p.tile([P, D], f32, tag="m0")
            nc.vector.tensor_scalar(m0, hp, 0.0, None, Alu.min)
            e0 = epi_p.tile([P, D], f32, tag="e0")
            nc.scalar.activation(e0, m0, Act.Exp)
            a_t = epi_p.tile([P, D], f32, tag="a_t")
            nc.vector.scalar_tensor_tensor(a_t, hp, 0.0, e0,
                                           Alu.max, Alu.add)
            nc.vector.tensor_add(a_t, a_t, xm1[:, i, :])
            st = epi_p.tile([P, 6], f32, tag="st")
            nc.vector.bn_stats(st, a_t)
            mv = epi_p.tile([P, 2], f32, tag="mv")
            nc.vector.bn_aggr(mv, st)
            sd = epi_p.tile([P, 1], f32, tag="sd")
            nc.scalar.activation(sd, mv[:, 1:2], Act.Sqrt, bias=eps_t)
            rstd = epi_p.tile([P, 1], f32, tag="rstd")
            nc.vector.reciprocal(rstd, sd)
            o_t = epi_p.tile([P, D], f32, tag="o_t")
            nc.vector.tensor_scalar(o_t, a_t, mv[:, 0:1], rstd,
                                    Alu.subtract, Alu.mult)
            nc.vector.tensor_mul(o_t, o_t, gam)
            nc.vector.tensor_add(o_t, o_t, bet)
            nc.sync.dma_start(out=out_d[i * P:(i + 1) * P, :], in_=o_t)

    nc.compile()
    return nc


def _prepare(x, prev_adj, log_sigmas, transition_gate, W, b, gamma, beta):
    import ml_dtypes
    bf16 = ml_dtypes.bfloat16

    x = np.asarray(x, np.float32)
    prev_adj = np.asarray(prev_adj, np.float32)
    log_sigmas = np.asarray(log_sigmas, np.float32)
    transition_gate = np.asarray(transition_gate, np.float32)
    W = np.asarray(W, np.float32)
    b = np.asarray(b, np.float32)
    gamma = np.asarray(gamma, np.float32)
    beta = np.asarray(beta, np.float32)

    gate = float(1.0 / (1.0 + math.exp(-float(transition_gate[0]))))
    sigmas = np.exp(log_sigmas.astype(np.float64))
    denoms = 2.0 * sigmas ** 2 + KERN_EPS          # [H]
    uden, counts = np.unique(denoms, return_counts=True)
    scales = tuple(float(2.0 / d) for d in uden)
    gcs = tuple(float(gate * c / H) for c in counts)

    sq = np.einsum("bnd,bnd->bn", x, x).astype(np.float32)   # [B, N]

    bump4 = np.zeros((4, P, QW), bf16)
    for v in range(4):
        for p in range(P):
            bump4[v, p, (v * P + p) % QW] = BUMP
    identb = np.eye(P, dtype=bf16)
    identf = np.eye(P, dtype=np.float32)
    WTb = np.ascontiguousarray(W.T).astype(bf16)
    gam_rep = np.ascontiguousarray(np.broadcast_to(gamma, (P, D)))
    bet_rep = np.ascontiguousarray(np.broadcast_to(beta, (P, D)))

    per_core = []
    for bi in range(B):
        biasv = np.stack([(-sq[bi] / np.float32(d)
                           + np.float32(math.log(g))).astype(np.float32)
                          for d, g in zip(uden, gcs)])
        per_core.append({
            "xTb": np.ascontiguousarray(x[bi].T).astype(bf16),
            "xm1": np.ascontiguousarray(x[bi] - 1.0),
            "prev": np.ascontiguousarray(prev_adj[bi]),
            "pvtb": np.ascontiguousarray(
                prev_adj[bi].T * np.float32(1.0 - gate)).astype(bf16),
            "negsq": (-sq[bi] / 2.0).astype(bf16),
            "biasv": np.ascontiguousarray(biasv),
            "WTb": WTb,
            "bump4": bump4,
            "identb": identb,
            "identf": identf,
            "bvec": b,
            "gamma_rep": gam_rep,
            "beta_rep": bet_rep,
        })
    return scales, gcs, gate, per_core


def kernel(x, prev_adj, log_sigmas, transition_gate, W, b, gamma, beta):
    from concourse import bass_utils

    scales, gcs, gate, per_core = _prepare(
        x, prev_adj, log_sigmas, transition_gate, W, b, gamma, beta)

    key = (scales, gcs, round(1.0 - gate, 9))
    if key not in _CACHE:
        _CACHE[key] = _build_program(scales, gcs, 1.0 - gate)
    nc = _CACHE[key]

    res = bass_utils.run_bass_kernel_spmd(nc, per_core,
                                          core_ids=list(range(B)))
    out = np.stack([r["out"] for r in res.results]).astype(np.float32)
    adj = np.stack([r["adj"] for r in res.results]).astype(np.float32)
    return out, adj


# revision 20
# speedup vs baseline: 1.1670x; 1.1670x over previous
"""MicrostateDGFLayer Trainium2 kernel.

Data-parallel over batch B=8 across 8 NeuronCores (one batch element per
core). Per core:
- pairwise sq-distances via bf16 PE matmuls (exact-diagonal trick: +8*I
  bump on the distance psum diagonal plus a min-clamp of the diagonal
  block pins adj[n,n] to its analytic value, making bf16 cancellation
  error there irrelevant; off-diagonal distances are O(100) so bf16
  noise vanishes under exp),
- per-head Gaussian kernel on ACT with gate/head-count folded into the
  exp bias (written directly as bf16 "eb" tiles),
- gated fusion with prev_adj on DVE (f32 adj output),
- graph propagation h = adj @ (x@W.T) with NO on-chip transposes: the
  Gaussian part of adj is symmetric, so its transposed blocks are other
  row-tiles' column slices (eb_j[:, i*128:...]); the asymmetric prev
  part streams host-pretransposed (1-gate)*prev.T in bf16. hT
  accumulates in a persistent [128 d, 2048 n] psum (4 banks), one
  weight load (xW_j) + 32 N=128 matmuls per row tile.
- tail: bias add, PE transpose back to row-major, ELU+residual+LN.
"""

import math
from contextlib import ExitStack

import numpy as np

B, N, D, H = 8, 2048, 128, 8
P = 128
NT = N // P          # 16 row tiles per core
QW = 512             # dist psum computed in [128, 512] quarters (1 bank)
LN_EPS = 1e-5
KERN_EPS = 1e-6
BUMP = 8.0           # diagonal psum bump; > any bf16 matmul error there

_CACHE: dict = {}


def _build_program(scales, gcs, one_minus_gate):
    """Build + compile the SPMD Bass program. scales[k] = 2/denom_k,
    gcs[k] = gate*count_k/H (both baked as instruction immediates)."""
    import concourse.bacc as bacc
    import concourse.tile as tile
    from concourse import mybir

    U = len(scales)
    f32 = mybir.dt.float32
    bf16 = mybir.dt.bfloat16
    Alu = mybir.AluOpType
    Act = mybir.ActivationFunctionType
    MMW = min(QW, N)
    import ml_dtypes
    gc_sum = float(sum(gcs))
    diag_delta = gc_sum - float(np.float32(ml_dtypes.bfloat16(gc_sum)))

    nc = bacc.Bacc("TRN2", target_bir_lowering=False, debug=False,
                   num_devices=B)

    xTb_d = nc.dram_tensor("xTb", [D, N], bf16, kind="ExternalInput").ap()
    xm1_d = nc.dram_tensor("xm1", [N, D], f32, kind="ExternalInput").ap()
    prev_d = nc.dram_tensor("prevb", [N, N], bf16, kind="ExternalInput").ap()
    pvtb_d = nc.dram_tensor("pvtb", [N, N], bf16, kind="ExternalInput").ap()
    negsq_d = nc.dram_tensor("negsq", [N], bf16, kind="ExternalInput").ap()
    biasv_d = nc.dram_tensor("biasv", [U, N], f32, kind="ExternalInput").ap()
    WTb_d = nc.dram_tensor("WTb", [D, D], bf16, kind="ExternalInput").ap()
    bump_d = nc.dram_tensor("bump4", [4, P, QW], bf16,
                            kind="ExternalInput").ap()
    idb_d = nc.dram_tensor("identb", [P, P], bf16, kind="ExternalInput").ap()
    idf_d = nc.dram_tensor("identf", [P, P], f32, kind="ExternalInput").ap()
    bvec_d = nc.dram_tensor("bvec", [D], f32, kind="ExternalInput").ap()
    gam_d = nc.dram_tensor("gamma_rep", [P, D], f32, kind="ExternalInput").ap()
    bet_d = nc.dram_tensor("beta_rep", [P, D], f32, kind="ExternalInput").ap()
    adj_d = nc.dram_tensor("adj", [N, N], f32, kind="ExternalOutput").ap()
    out_d = nc.dram_tensor("out", [N, D], f32, kind="ExternalOutput").ap()

    with tile.TileContext(nc) as tc, ExitStack() as ctx:
        consts = ctx.enter_context(tc.tile_pool(name="consts", bufs=1))
        prev_p = ctx.enter_context(tc.tile_pool(name="prev", bufs=3))
        pvtb_p = ctx.enter_context(tc.tile_pool(name="pvtb", bufs=3))
        adj_p = ctx.enter_context(tc.tile_pool(name="adjp", bufs=3))
        eb_p = ctx.enter_context(tc.tile_pool(name="ebp", bufs=3))
        htb_p = ctx.enter_context(tc.tile_pool(name="htb", bufs=2))
        epi_p = ctx.enter_context(tc.tile_pool(name="epi", bufs=3))
        tail_p = ctx.enter_context(tc.tile_pool(name="tail", bufs=1))
        ps_dist = ctx.enter_context(
            tc.tile_pool(name="ps_dist", bufs=3, space="PSUM"))
        ps_h = ctx.enter_context(
            tc.tile_pool(name="ps_h", bufs=1, space="PSUM"))
        ps_tph = ctx.enter_context(
            tc.tile_pool(name="ps_tph", bufs=1, space="PSUM"))

        # ---- constants ----
        xTb = consts.tile([D, N], bf16)
        nc.sync.dma_start(out=xTb, in_=xTb_d)
        WTb = consts.tile([D, D], bf16)
        nc.sync.dma_start(out=WTb, in_=WTb_d)
        xm1 = consts.tile([P, NT, D], f32)
        nc.sync.dma_start(out=xm1, in_=xm1_d.rearrange("(j p) d -> p j d", p=P))
        negsq = consts.tile([1, N], bf16)
        nc.sync.dma_start(out=negsq, in_=negsq_d.rearrange("(o n) -> o n", o=1))
        biasv = consts.tile([P, U, NT], f32)
        nc.sync.dma_start(out=biasv,
                          in_=biasv_d.rearrange("u (j p) -> p u j", p=P))
        bump = consts.tile([P, 4, QW], bf16)
        nc.sync.dma_start(out=bump, in_=bump_d.rearrange("v p n -> p v n"))
        identb = consts.tile([P, P], bf16)
        nc.sync.dma_start(out=identb, in_=idb_d)
        identf = consts.tile([P, P], f32)
        nc.sync.dma_start(out=identf, in_=idf_d)
        bvec = consts.tile([D, 1], f32)
        nc.sync.dma_start(out=bvec, in_=bvec_d.rearrange("(d o) -> d o", o=1))
        gam = consts.tile([P, D], f32)
        nc.sync.dma_start(out=gam, in_=gam_d)
        bet = consts.tile([P, D], f32)
        nc.sync.dma_start(out=bet, in_=bet_d)
        ones1 = consts.tile([1, P], bf16)
        nc.vector.memset(ones1, 1.0)
        eps_t = consts.tile([P, 1], f32)
        nc.vector.memset(eps_t, LN_EPS)

        # ---- xWb = bf16(x @ W.T), row-major [m, d] chunks ----
        xWb = consts.tile([P, N], bf16)
        for mc in range(NT):
            pt = ps_dist.tile([P, P], f32, tag="ps")
            nc.tensor.matmul(pt, xTb[:, mc * P:(mc + 1) * P], WTb,
                             start=True, stop=True)
            nc.scalar.activation(xWb[:, mc * P:(mc + 1) * P], pt, Act.Copy)

        # persistent hT accumulator [d, n] over the whole row sweep
        ht = ps_h.tile([P, N], f32)

        for j in range(NT):
            rows = slice(j * P, (j + 1) * P)
            gd = j * P                      # diagonal block column offset
            prev_t = prev_p.tile([P, N], bf16)
            nc.sync.dma_start(out=prev_t, in_=prev_d[rows, :])
            pvtb_t = pvtb_p.tile([P, N], bf16)
            nc.sync.dma_start(out=pvtb_t, in_=pvtb_d[rows, :])
            adj_t = adj_p.tile([P, N], f32)
            eb_t = eb_p.tile([P, N], bf16)

            for q in range(N // MMW):
                c0 = q * MMW
                ps = ps_dist.tile([P, MMW], f32, tag="ps")
                # psum = x_j@x.T - sq_m/2 (+BUMP on diag); ACT scale doubles
                nc.tensor.matmul(ps, xTb[:, rows], xTb[:, c0:c0 + MMW],
                                 start=True, stop=False)
                has_diag = c0 <= gd < c0 + MMW
                nc.tensor.matmul(ps, ones1, negsq[:, c0:c0 + MMW],
                                 start=False, stop=not has_diag)
                if has_diag:
                    v = (gd % MMW) // P
                    nc.tensor.matmul(ps, identb, bump[:, v, :MMW],
                                     start=False, stop=True)
                cols = slice(c0, c0 + MMW)
                if U == 1:
                    nc.scalar.activation(eb_t[:, cols], ps, Act.Exp,
                                         bias=biasv[:, 0, j:j + 1],
                                         scale=scales[0])
                else:
                    accf = epi_p.tile([P, MMW], f32, tag="accf")
                    for u in range(U):
                        e_u = epi_p.tile([P, MMW], f32, tag="e_u")
                        dst = accf if u == 0 else e_u
                        nc.scalar.activation(dst, ps, Act.Exp,
                                             bias=biasv[:, u, j:j + 1],
                                             scale=scales[u])
                        if has_diag:
                            dc = gd % MMW
                            nc.vector.tensor_scalar(dst[:, dc:dc + P],
                                                    dst[:, dc:dc + P],
                                                    gcs[u], None, Alu.min)
                        if u > 0:
                            nc.vector.tensor_add(accf, accf, e_u)
                    nc.vector.tensor_copy(eb_t[:, cols], accf)
                if U == 1 and has_diag:
                    dc = gd % MMW
                    nc.vector.tensor_scalar(eb_t[:, gd:gd + P],
                                            eb_t[:, gd:gd + P],
                                            gcs[0], None, Alu.min)
                # adj = prev_scaled + eb  (host pre-scaled (1-g)*prev)
                nc.vector.tensor_add(adj_t[:, cols], prev_t[:, cols],
                                     eb_t[:, cols])

            # eb's diagonal is bf16(sum gc) — restore the exact f32 value
            # on adj's diagonal with a delta*I correction.
            nc.vector.scalar_tensor_tensor(
                adj_t[:, gd:gd + P], identf, diag_delta,
                adj_t[:, gd:gd + P], Alu.mult, Alu.add)
            nc.sync.dma_start(out=adj_d[rows, :], in_=adj_t)

            # hT[d, n] += xW_j.T @ (sym-part + prevT-part) for chunk j.
            # eb_j[:, slab] IS the transposed block of the symmetric
            # Gaussian part; pvtb_j is host-pretransposed (1-g)*prev.T.
            # One [128,512] slab per psum bank, single stationary xW_j.
            for sl in range(N // MMW):
                ssl = slice(sl * MMW, (sl + 1) * MMW)
                nc.tensor.matmul(ht[:, ssl], xWb[:, rows], eb_t[:, ssl],
                                 start=(j == 0), stop=False)
                nc.tensor.matmul(ht[:, ssl], xWb[:, rows], pvtb_t[:, ssl],
                                 start=False, stop=(j == NT - 1))

        # ---- tail: bias, transpose back, ELU + residual + LayerNorm ----
        # Three phases so ACT doesn't thrash activation tables
        # (Identity/Exp in phase 1, all Sqrts in phase 2).
        a_ts, mvs = [], []
        for i in range(NT):
            isl = slice(i * P, (i + 1) * P)
            htb = htb_p.tile([P, P], f32)
            nc.scalar.activation(htb, ht[:, isl], Act.Identity, bias=bvec)
            hp = ps_tph.tile([P, P], f32, tag="tph")
            nc.tensor.transpose(hp, htb, identf)
            # a = relu(h) + exp(min(h,0)) + (x-1)  == elu(h) + x
            m0 = epi_p.tile([P, D], f32, tag="m0")
            nc.vector.tensor_scalar(m0, hp, 0.0, None, Alu.min)
            e0 = epi_p.tile([P, D], f32, tag="e0")
            nc.scalar.activation(e0, m0, Act.Exp)
            a_t = tail_p.tile([P, D], f32, tag=f"a_t{i}")
            nc.vector.scalar_tensor_tensor(a_t, hp, 0.0, e0,
                                           Alu.max, Alu.add)
            nc.vector.tensor_add(a_t, a_t, xm1[:, i, :])
            st = epi_p.tile([P, 6], f32, tag="st")
            nc.vector.bn_stats(st, a_t)
            mv = tail_p.tile([P, 2], f32, tag=f"mv{i}")
            nc.vector.bn_aggr(mv, st)
            a_ts.append(a_t)
            mvs.append(mv)
        sds = []
        for i in range(NT):
            sd = tail_p.tile([P, 1], f32, tag=f"sd{i}")
            nc.scalar.activation(sd, mvs[i][:, 1:2], Act.Sqrt, bias=eps_t)
            sds.append(sd)
        for i in range(NT):
            rstd = epi_p.tile([P, 1], f32, tag="rstd")
            nc.vector.reciprocal(rstd, sds[i])
            o_t = epi_p.tile([P, D], f32, tag="o_t")
            nc.vector.tensor_scalar(o_t, a_ts[i], mvs[i][:, 0:1], rstd,
                                    Alu.subtract, Alu.mult)
            nc.vector.tensor_mul(o_t, o_t, gam)
            nc.vector.tensor_add(o_t, o_t, bet)
            nc.sync.dma_start(out=out_d[i * P:(i + 1) * P, :], in_=o_t)

    nc.compile()
    return nc


def _prepare(x, prev_adj, log_sigmas, transition_gate, W, b, gamma, beta):
    import ml_dtypes
    bf16 = ml_dtypes.bfloat16

    x = np.asarray(x, np.float32)
    prev_adj = np.asarray(prev_adj, np.float32)
    log_sigmas = np.asarray(log_sigmas, np.float32)
    transition_gate = np.asarray(transition_gate, np.float32)
    W = np.asarray(W, np.float32)
    b = np.asarray(b, np.float32)
    gamma = np.asarray(gamma, np.float32)
    beta = np.asarray(beta, np.float32)

    gate = float(1.0 / (1.0 + math.exp(-float(transition_gate[0]))))
    sigmas = np.exp(log_sigmas.astype(np.float64))
    denoms = 2.0 * sigmas ** 2 + KERN_EPS          # [H]
    uden, counts = np.unique(denoms, return_counts=True)
    scales = tuple(float(2.0 / d) for d in uden)
    gcs = tuple(float(gate * c / H) for c in counts)

    sq = np.einsum("bnd,bnd->bn", x, x).astype(np.float32)   # [B, N]

    bump4 = np.zeros((4, P, QW), bf16)
    for v in range(4):
        for p in range(P):
            bump4[v, p, (v * P + p) % QW] = BUMP
    identb = np.eye(P, dtype=bf16)
    identf = np.eye(P, dtype=np.float32)
    WTb = np.ascontiguousarray(W.T).astype(bf16)
    gam_rep = np.ascontiguousarray(np.broadcast_to(gamma, (P, D)))
    bet_rep = np.ascontiguousarray(np.broadcast_to(beta, (P, D)))

    per_core = []
    for bi in range(B):
        biasv = np.stack([(-sq[bi] / np.float32(d)
                           + np.float32(math.log(g))).astype(np.float32)
                          for d, g in zip(uden, gcs)])
        per_core.append({
            "xTb": np.ascontiguousarray(x[bi].T).astype(bf16),
            "xm1": np.ascontiguousarray(x[bi] - 1.0),
            "prevb": (prev_adj[bi] * np.float32(1.0 - gate)).astype(bf16),
            "pvtb": np.ascontiguousarray(
                prev_adj[bi].T * np.float32(1.0 - gate)).astype(bf16),
            "negsq": (-sq[bi] / 2.0).astype(bf16),
            "biasv": np.ascontiguousarray(biasv),
            "WTb": WTb,
            "bump4": bump4,
            "identb": identb,
            "identf": identf,
            "bvec": b,
            "gamma_rep": gam_rep,
            "beta_rep": bet_rep,
        })
    return scales, gcs, gate, per_core


def kernel(x, prev_adj, log_sigmas, transition_gate, W, b, gamma, beta):
    from concourse import bass_utils

    scales, gcs, gate, per_core = _prepare(
        x, prev_adj, log_sigmas, transition_gate, W, b, gamma, beta)

    key = (scales, gcs, round(1.0 - gate, 9))
    if key not in _CACHE:
        _CACHE[key] = _build_program(scales, gcs, 1.0 - gate)
    nc = _CACHE[key]

    res = bass_utils.run_bass_kernel_spmd(nc, per_core,
                                          core_ids=list(range(B)))
    out = np.stack([r["out"] for r in res.results]).astype(np.float32)
    adj = np.stack([r["adj"] for r in res.results]).astype(np.float32)
    return out, adj
